# revision 11
# baseline (speedup 1.0000x reference)
"""Trainium2 Bass kernel for nn_MiniLSTM (2-layer LSTM LM).

Model: embed[x] -> LSTM0 -> LSTM1 -> LayerNorm -> vocab projection.
Shapes: B=64, T=1024, E=128, H=256, V=2048.

Design (single SPMD launch, 8 cores, one identical program, no collectives):
  - The LSTM recurrences (the serial bottleneck) run redundantly on every
    core in "orientation B": all per-step tensors are kept transposed as
    [dim-on-partitions, batch-on-free] tiles so no per-step transposes are
    needed and the elementwise chain uses all 128 lanes.
  - Layer 0 and layer 1 are interleaved step-by-step (layer 1 lags one
    chunk) so each layer's elementwise tail hides under the other layer's
    matmuls.
  - g0 (embedding + input projection of layer 0) is a row-gather from a
    host-prepacked table:  table0[v] = embed[v] @ W_ih0.T + b_ih0 + b_hh0
    (weight-only prepack), fetched with indirect DMA.
  - xg enters PSUM via identity matmuls; W_hh tiles are bf16 stationaries.
  - LayerNorm is folded algebraically into the output projection:
      logits = r * (h @ Wg^T + (-mu)*sg + s*bbeta),  Wg = W_out * ln_g,
      sg = Wg.sum(-1), bbeta = ln_b @ W_out.T + b_out, s = sqrt(var+eps),
      r = 1/s.  The (-mu, s) terms ride along as 2 extra contraction rows.
  - The output projection is sharded over the vocab dim: core c gets rows
    [256c, 256c+256) of W_out (different input DATA, same program).  Host
    concatenates the logit shards.
"""

import os
import sys

for _p in ("/opt/trn_rl_repo", "/root/.axon_site/_ro/trn_rl_repo"):
    if os.path.isdir(_p) and _p not in sys.path:
        sys.path.insert(0, _p)

import numpy as np
import ml_dtypes

import concourse.bass as bass
import concourse.bacc as bacc
import concourse.mybir as mybir
import concourse.tile as tile

B = 64
T = 1024
E = 128
H = 256
V = 2048
G = 4 * H  # 1024 gate pre-activations
P = 128
NCORES = 8
VSHARD = V // NCORES  # 256
LN_EPS = 1e-5

# PSUM gate-chunk c holds W rows [128*BLOCK[c], 128*BLOCK[c]+128).
# Order: i0 i1 f0 f1 o0 o1 g0 g1  (sigmoid gates first 6, tanh last 2).
BLOCK = [0, 1, 2, 3, 6, 7, 4, 5]

F32 = mybir.dt.float32
BF16 = mybir.dt.bfloat16
I32 = mybir.dt.int32

bf16 = ml_dtypes.bfloat16
AF = mybir.ActivationFunctionType


def _perm_rows(w):
    """Permute gate rows [4H, ...] into BLOCK chunk order."""
    blocks = w.reshape(8, P, *w.shape[1:])
    return np.concatenate([blocks[b][None] for b in BLOCK], axis=0).reshape(w.shape)


def _pack_tiles_pm(w):
    """[4H, K] weight -> partition-major stationary tiles [128(k), 8(gc),
    K//128(kc), 128(m)] with tile[k, gc, kc, m] = w_perm[128*gc+m, 128*kc+k]."""
    gperm = _perm_rows(np.asarray(w, np.float32))  # [1024, K]
    K = gperm.shape[1]
    nk = K // P
    out = np.empty((P, 8, nk, P), dtype=np.float32)
    for gc in range(8):
        for kc in range(nk):
            blockw = gperm[gc * P:(gc + 1) * P, kc * P:(kc + 1) * P]  # [m, k]
            out[:, gc, kc, :] = blockw.T  # [k, m]
    return out


def build_nc(T_=T, S=16, vshard=VSHARD, g0_bufs=12):
    """Build the Bass program (identical on all cores)."""
    K = T_ // S  # number of chunks
    TOK = S * B  # tokens per chunk
    NT = TOK // 512  # 512-token n-tiles per chunk for GEMMs
    TT = TOK // P  # 128-token tiles per chunk for out-proj

    nc = bacc.Bacc("TRN2", target_bir_lowering=False, num_devices=NCORES)

    # ---- kernel I/O -------------------------------------------------------
    table0p = nc.declare_dram_parameter("table0p", [V, G], BF16, isOutput=False)
    ids = nc.declare_dram_parameter("ids", [P, T_ // 2], I32, isOutput=False)
    whh0t = nc.declare_dram_parameter("whh0t", [P, 8, 2, P], BF16, isOutput=False)
    whh1t = nc.declare_dram_parameter("whh1t", [P, 8, 2, P], BF16, isOutput=False)
    wih1t = nc.declare_dram_parameter("wih1t", [P, 8, 2, P], BF16, isOutput=False)
    b1p = nc.declare_dram_parameter("b1p", [P, 8], F32, isOutput=False)
    i64 = nc.declare_dram_parameter("i64", [2 * B, B], BF16, isOutput=False)
    i128 = nc.declare_dram_parameter("i128", [P, P], BF16, isOutput=False)
    onesk = nc.declare_dram_parameter("onesk", [P, 1], BF16, isOutput=False)
    wgt = nc.declare_dram_parameter("wgt", [P, 2, vshard], BF16, isOutput=False)
    sgrow = nc.declare_dram_parameter("sgrow", [1, vshard], BF16, isOutput=False)
    bbrow = nc.declare_dram_parameter("bbrow", [1, vshard], BF16, isOutput=False)

    logits_o = nc.declare_dram_parameter("logits_o", [B, T_, vshard], F32, isOutput=True)
    h0t_o = nc.declare_dram_parameter("h0t_o", [P, 2, B], F32, isOutput=True)
    c0_o = nc.declare_dram_parameter("c0_o", [P, 2, B], F32, isOutput=True)
    h1t_o = nc.declare_dram_parameter("h1t_o", [P, 2, B], F32, isOutput=True)
    c1_o = nc.declare_dram_parameter("c1_o", [P, 2, B], F32, isOutput=True)

    with tile.TileContext(nc) as tc:
        with (
            tc.tile_pool(name="const", bufs=1) as constp,
            tc.tile_pool(name="state", bufs=1) as statep,
            tc.tile_pool(name="g0ring", bufs=g0_bufs) as g0ring,
            tc.tile_pool(name="hbuf", bufs=2) as hbufp,
            tc.tile_pool(name="g1buf", bufs=2) as g1bufp,
            tc.tile_pool(name="ew", bufs=4) as ewp,
            tc.tile_pool(name="rows", bufs=1) as rowsp,
            tc.tile_pool(name="logsb", bufs=4) as logsbp,
            tc.tile_pool(name="recps", bufs=3, space="PSUM") as recps,
            tc.tile_pool(name="g1ps", bufs=2, space="PSUM") as g1ps,
            tc.tile_pool(name="opps", bufs=1, space="PSUM") as opps,
            tc.tile_pool(name="stps", bufs=2, space="PSUM") as stps,
            tc.tile_pool(name="dram", bufs=2, space="DRAM") as dramp,
        ):
            # ---- load constants into SBUF --------------------------------
            ids_sb = constp.tile([P, T_ // 2], I32, tag="ids")
            b1_sb = constp.tile([P, 8], F32, tag="b1")
            i64_sb = constp.tile([2 * B, B], BF16, tag="i64")
            i128_sb = constp.tile([P, P], BF16, tag="i128")
            ones_sb = constp.tile([P, 1], BF16, tag="ones")
            wgt_sb = constp.tile([P, 2, vshard], BF16, tag="wgt")
            sg_sbuf = constp.tile([1, vshard], BF16, tag="sgrow")
            bb_sbuf = constp.tile([1, vshard], BF16, tag="bbrow")
            w0t = constp.tile([P, 8, 2, P], BF16, tag="w0t")
            w1t = constp.tile([P, 8, 2, P], BF16, tag="w1t")
            wi1 = constp.tile([P, 8, 2, P], BF16, tag="wi1t")

            nc.sync.dma_start(out=ids_sb[:], in_=ids[:])
            nc.sync.dma_start(out=b1_sb[:], in_=b1p[:])
            nc.sync.dma_start(out=i64_sb[:], in_=i64[:])
            nc.sync.dma_start(out=i128_sb[:], in_=i128[:])
            nc.sync.dma_start(out=ones_sb[:], in_=onesk[:])
            nc.sync.dma_start(out=wgt_sb[:], in_=wgt[:])
            nc.sync.dma_start(out=sg_sbuf[:], in_=sgrow[:])
            nc.sync.dma_start(out=bb_sbuf[:], in_=bbrow[:])
            nc.sync.dma_start(out=w0t[:], in_=whh0t[:])
            nc.sync.dma_start(out=w1t[:], in_=whh1t[:])
            nc.sync.dma_start(out=wi1[:], in_=wih1t[:])

            # ---- state ----------------------------------------------------
            c_init = statep.tile([P, 2, B], F32, tag="cini")
            h_init = statep.tile([P, 2, B], BF16, tag="hini")
            nc.vector.memset(c_init[:], 0.0)
            nc.vector.memset(h_init[:], 0.0)
            c_state = {0: c_init[:], 1: c_init[:]}

            g0_tiles = {}  # global 2-step-tile index -> sbuf tile

            def emit_gather(gt):
                t0 = g0ring.tile([P, G], BF16, tag="g0")
                nc.gpsimd.indirect_dma_start(
                    out=t0[:],
                    out_offset=None,
                    in_=table0p[:],
                    in_offset=bass.IndirectOffsetOnAxis(ap=ids_sb[:, gt:gt + 1], axis=0),
                )
                g0_tiles[gt] = t0

            def rec_step(layer, h_prev, h_out_ap, xg_lhs=None, g1src=None,
                         xg_half=0):
                """One LSTM step in orientation B.

                h_prev/h_out_ap: AP [128, 2, 64] bf16 (h^T tiles)
                xg_lhs: layer0 — stationary AP [64, 1024] (gathered g0 rows)
                g1src: layer1 — AP [128, 8, 64]-view bf16 (g1^T for this step)
                """
                w_sb = w0t if layer == 0 else w1t
                cprev = c_state[layer]
                pt = recps.tile([P, 8, B], F32, tag="rec")

                # xg into PSUM. start=True on the FIRST matmul into the
                # bank clears has_written for the whole bank; all later MMs
                # accumulate (first touch of each region overwrites).
                if layer == 0:
                    for gc in range(8):
                        nc.tensor.matmul(
                            pt[:, gc, :],
                            xg_lhs[:, gc * P:(gc + 1) * P],
                            i64_sb[xg_half * B:(xg_half + 1) * B, :],
                            start=(gc == 0), stop=False,
                        )
                else:
                    for gc in range(8):
                        nc.tensor.matmul(
                            pt[:, gc, :],
                            i128_sb[:],
                            g1src[:, gc, :],
                            start=(gc == 0), stop=False,
                        )
                # recurrent weight matmuls
                for gc in range(8):
                    for kc in range(2):
                        nc.tensor.matmul(
                            pt[:, gc, :],
                            w_sb[:, gc, kc, :],
                            h_prev[:, kc, :],
                            start=False, stop=(gc == 7 and kc == 1),
                        )

                # elementwise
                sig6 = ewp.tile([P, 6, B], BF16, tag=f"sig{layer}")
                tg = ewp.tile([P, 2, B], BF16, tag=f"tg{layer}")
                tmp = ewp.tile([P, 2, B], BF16, tag=f"tmp{layer}")
                tcl = ewp.tile([P, 2, B], BF16, tag=f"tc{layer}")
                cnew = ewp.tile([P, 2, B], F32, tag=f"c{layer}")
                nc.scalar.activation(sig6[:], pt[:, 0:6, :], AF.Sigmoid)
                nc.scalar.activation(tg[:], pt[:, 6:8, :], AF.Tanh)
                nc.vector.tensor_mul(tmp[:], sig6[:, 0:2, :], tg[:])      # i*g~
                nc.vector.tensor_mul(cnew[:], sig6[:, 2:4, :], cprev)     # f*c
                nc.vector.tensor_add(cnew[:], cnew[:], tmp[:])            # c new
                nc.scalar.activation(tcl[:], cnew[:], AF.Tanh)
                nc.vector.tensor_mul(h_out_ap, sig6[:, 4:6, :], tcl[:])   # h=o*tanh(c)
                c_state[layer] = cnew[:]

            # prefetch gathers for chunk 0
            for gt in range(S // 2):
                emit_gather(gt)

            h0bufs = {}
            h1bufs = {}
            g1bufs = {}

            h0_prev = h_init[:]
            h1_prev = h_init[:]

            for k in range(K + 1):
                do_l0 = k < K
                do_l1 = k >= 1

                if do_l0:
                    h0buf = hbufp.tile([P, 2, TOK], BF16, tag="h0buf")
                    h0bufs[k] = h0buf
                if do_l1:
                    h1buf = hbufp.tile([P, 2, TOK], BF16, tag="h1buf")
                    h1bufs[k - 1] = h1buf

                for s in range(S):
                    ssl = slice(s * B, (s + 1) * B)
                    if do_l0:
                        t = k * S + s
                        gt, half = t // 2, t % 2
                        xg_lhs = g0_tiles[gt][half * B:(half + 1) * B, :]
                        rec_step(0, h0_prev, h0buf[:, :, ssl], xg_lhs=xg_lhs,
                                 xg_half=half)
                        h0_prev = h0buf[:, :, ssl]
                        if s % 2 == 0 and k + 1 < K:
                            emit_gather(((k + 1) * S) // 2 + s // 2)
                    if do_l1:
                        g1sb = g1bufs[k - 1]
                        rec_step(1, h1_prev, h1buf[:, :, ssl],
                                 g1src=g1sb[:, :, ssl])
                        h1_prev = h1buf[:, :, ssl]

                # ---- g1 = h0_chunk @ W_ih1^T + b1 (for layer1, next iter) --
                if do_l0:
                    g1sb = g1bufp.tile([P, 8, TOK], BF16, tag="g1")
                    g1bufs[k] = g1sb
                    for nt in range(NT):
                        nsl = slice(nt * 512, (nt + 1) * 512)
                        for gc in range(8):
                            pg = g1ps.tile([P, 512], F32, tag="g1p")
                            for kc in range(2):
                                nc.tensor.matmul(
                                    pg[:],
                                    wi1[:, gc, kc, :],
                                    h0buf[:, kc, nsl],
                                    start=(kc == 0), stop=(kc == 1),
                                )
                            # evacuate with per-partition bias, alternate engines
                            if gc % 2 == 0:
                                nc.scalar.activation(
                                    g1sb[:, gc, nsl], pg[:], AF.Identity,
                                    bias=b1_sb[:, gc:gc + 1],
                                )
                            else:
                                nc.vector.tensor_scalar_add(
                                    g1sb[:, gc, nsl], pg[:], b1_sb[:, gc:gc + 1],
                                )

                # ---- LayerNorm-folded out-projection for h1 chunk k-1 ------
                if do_l1:
                    j = k - 1  # chunk index of h1 produced this iteration
                    h1f = h1buf[:]  # [P, 2, TOK]
                    hsq = rowsp.tile([P, 2, TOK], BF16, tag="hsq")
                    nc.vector.tensor_mul(hsq[:], h1f, h1f)
                    mu_sb = rowsp.tile([1, TOK], F32, tag="mu")
                    sq_sb = rowsp.tile([1, TOK], F32, tag="sq")
                    for nt in range(NT):
                        nsl = slice(nt * 512, (nt + 1) * 512)
                        pmu = stps.tile([1, 512], F32, tag="st")
                        psq = stps.tile([1, 512], F32, tag="st")
                        for kc in range(2):
                            nc.tensor.matmul(pmu[:], ones_sb[:], h1f[:, kc, nsl],
                                             start=(kc == 0), stop=(kc == 1))
                        for kc in range(2):
                            nc.tensor.matmul(psq[:], ones_sb[:], hsq[:, kc, nsl],
                                             start=(kc == 0), stop=(kc == 1))
                        nc.scalar.copy(mu_sb[:, nsl], pmu[:])
                        nc.scalar.copy(sq_sb[:, nsl], psq[:])
                    musq = rowsp.tile([1, TOK], F32, tag="musq")
                    var = rowsp.tile([1, TOK], F32, tag="var")
                    s_sb = rowsp.tile([1, TOK], F32, tag="srow")
                    nc.scalar.square(musq[:], mu_sb[:])
                    # var + eps = (sq + eps) - mu^2, one fused DVE op
                    nc.vector.scalar_tensor_tensor(
                        out=var[:], in0=sq_sb[:], scalar=LN_EPS, in1=musq[:],
                        op0=mybir.AluOpType.add, op1=mybir.AluOpType.subtract)
                    nc.scalar.activation(s_sb[:], var[:], AF.Sqrt)
                    negmu_bf = rowsp.tile([1, TOK], BF16, tag="negmu")
                    s_bf = rowsp.tile([1, TOK], BF16, tag="sbf")
                    nc.scalar.mul(negmu_bf[:], mu_sb[:], -1.0)
                    nc.scalar.copy(s_bf[:], s_sb[:])
                    # r = 1/s via HBM bounce into [128, TOK//128]
                    sb_d = dramp.tile([P, TOK // P], F32, tag="sbounce")
                    nc.sync.dma_start(
                        out=sb_d[:].rearrange("p t -> () t p"),
                        in_=s_sb[:].rearrange("a (t p) -> a t p", p=P),
                    )
                    r128 = rowsp.tile([P, TOK // P], F32, tag="r128")
                    nc.sync.dma_start(out=r128[:], in_=sb_d[:])
                    nc.vector.reciprocal(r128[:], r128[:])

                    for tt in range(TT):
                        pG = opps.tile([P, vshard], F32, tag="op")
                        tsl = slice(tt * P, (tt + 1) * P)
                        for kc in range(2):
                            nc.tensor.matmul(
                                pG[:],
                                h1f[:, kc, tsl],
                                wgt_sb[:, kc, :],
                                start=(kc == 0), stop=False,
                            )
                        nc.tensor.matmul(pG[:], negmu_bf[:, tsl], sg_sbuf[:],
                                         start=False, stop=False)
                        nc.tensor.matmul(pG[:], s_bf[:, tsl], bb_sbuf[:],
                                         start=False, stop=True)
                        lt = logsbp.tile([P, vshard], F32, tag="log")
                        nc.scalar.mul(lt[:], pG[:], r128[:, tt:tt + 1])
                        tg0 = j * S + (tt * P) // B
                        nc.sync.dma_start(out=logits_o[:, tg0, :], in_=lt[0:B, :])
                        nc.sync.dma_start(out=logits_o[:, tg0 + 1, :], in_=lt[B:2 * B, :])

            # ---- final states --------------------------------------------
            hof = statep.tile([P, 2, B], F32, tag="hof")
            nc.vector.tensor_copy(hof[:], h0bufs[K - 1][:, :, (S - 1) * B:S * B])
            nc.sync.dma_start(out=h0t_o[:], in_=hof[:])
            h1fo = statep.tile([P, 2, B], F32, tag="h1fo")
            nc.vector.tensor_copy(h1fo[:], h1bufs[K - 1][:, :, (S - 1) * B:S * B])
            nc.sync.dma_start(out=h1t_o[:], in_=h1fo[:])
            nc.sync.dma_start(out=c0_o[:], in_=c_state[0])
            nc.sync.dma_start(out=c1_o[:], in_=c_state[1])

    nc.compile()
    return nc


def prepare_inputs(x, embed, W_ih0, W_hh0, b_ih0, b_hh0, W_ih1, W_hh1, b_ih1,
                   b_hh1, ln_g, ln_b, W_out, b_out, T_=T, vshard=VSHARD,
                   ncores=NCORES):
    """Host-side weight prepacking. Returns per-core in_maps."""
    x = np.asarray(x)
    f32 = np.float32

    # gather table: embed @ W_ih0.T + biases, columns in BLOCK-permuted order
    table0 = np.asarray(embed, f32) @ np.asarray(W_ih0, f32).T \
        + np.asarray(b_ih0, f32) + np.asarray(b_hh0, f32)  # [V, 1024]
    table0p = _perm_rows(table0.T).T.copy().astype(bf16)

    # token ids arranged for 2-step gather tiles
    idsm = np.empty((P, T_ // 2), np.int32)
    for gt in range(T_ // 2):
        idsm[0:B, gt] = x[:, 2 * gt]
        idsm[B:P, gt] = x[:, 2 * gt + 1]

    whh0t = _pack_tiles_pm(W_hh0).astype(bf16)
    whh1t = _pack_tiles_pm(W_hh1).astype(bf16)
    wih1t = _pack_tiles_pm(W_ih1).astype(bf16)

    b1 = (np.asarray(b_ih1, f32) + np.asarray(b_hh1, f32))  # [1024]
    b1p = np.ascontiguousarray(_perm_rows(b1).reshape(8, P).T).astype(f32)

    i64m = np.concatenate([np.eye(B, dtype=np.float32)] * 2, axis=0).astype(bf16)
    i128m = np.eye(P, dtype=np.float32).astype(bf16)
    onesk = np.full((P, 1), 1.0 / H, dtype=np.float32).astype(bf16)

    Wg = np.asarray(W_out, f32) * np.asarray(ln_g, f32)[None, :]  # [V, H]
    sg = Wg.sum(axis=1)  # [V]
    bbeta = np.asarray(ln_b, f32) @ np.asarray(W_out, f32).T + np.asarray(b_out, f32)

    base = dict(
        table0p=table0p, ids=idsm, whh0t=whh0t, whh1t=whh1t, wih1t=wih1t,
        b1p=b1p, i64=i64m, i128=i128m, onesk=onesk,
    )
    in_maps = []
    for c in range(ncores):
        vsl = slice(c * vshard, (c + 1) * vshard)
        wgt_c = np.empty((P, 2, vshard), np.float32)
        for kc in range(2):
            wgt_c[:, kc, :] = Wg[vsl, kc * P:(kc + 1) * P].T
        m = dict(base)
        m["wgt"] = wgt_c.astype(bf16)
        m["sgrow"] = sg[vsl][None, :].astype(bf16)
        m["bbrow"] = bbeta[vsl][None, :].astype(bf16)
        in_maps.append(m)
    return in_maps


def assemble_outputs(results):
    """Combine per-core results into (logits, (h, c)) like the reference."""
    logits = np.concatenate(
        [np.asarray(r["logits_o"], np.float32).reshape(B, -1, VSHARD)
         for r in results], axis=-1)
    r0 = results[0]

    def unT(a):  # [128, 2, 64] -> [64, 256]
        a = np.asarray(a, np.float32).reshape(P, 2, B)
        return np.ascontiguousarray(np.transpose(a, (2, 1, 0)).reshape(B, H))

    h = np.stack([unT(r0["h0t_o"]), unT(r0["h1t_o"])])
    c = np.stack([unT(r0["c0_o"]), unT(r0["c1_o"])])
    return logits, (h, c)


_NC_CACHE = {}


def kernel(**inputs):
    from concourse import bass_utils

    in_maps = prepare_inputs(**inputs)
    if "main" not in _NC_CACHE:
        _NC_CACHE["main"] = build_nc()
    nc = _NC_CACHE["main"]
    res = bass_utils.run_bass_kernel_spmd(nc, in_maps, core_ids=list(range(NCORES)))
    return assemble_outputs(res.results)


if __name__ == "__main__":
    nc = build_nc(T_=64, S=16)
    print("built ok")
